# revision 29
# baseline (speedup 1.0000x reference)
"""Trainium2 Bass kernel for nn_Encoder_21715354649023.

Math: tokens -> emb gather -> xw = x@W -> GRU-like scan over T:
    pre = xw_t + h@U ; z = sigmoid(pre+b_g) ; c = tanh(pre+b_h)
    h' = z*h + (mix*(1-z)+nu_s)*c,  mix = sigmoid(zeta), nu_s = sigmoid(nu)

Device strategy (data-parallel over batch, 32 rows/core on 8 cores):
  * r-trick: pre = (h + r_t) @ U with r_t = (xw_t) @ U^-1.  The host
    precomputes EM = emb @ (W @ U^-1) once and gathers r rows by token, so
    the per-step xw injection becomes a second accumulating matmul with the
    SAME stationary U (no PSUM preload, which DMA cannot do anyway).
  * segment parallelism: T=512 is split into S segments per core, each
    warmed up W steps from h=0 (the sigmoid gate contracts the state, so
    initial-state influence decays ~e^{-0.066 k}).  This turns 512 serial
    steps into (T/S + W) wider steps whose per-instruction overheads
    amortize across S*32 lanes.
  * per step, per lane-group: 2 matmuls (U.T@r acc U.T@h) -> PSUM;
    tanh+sigmoid on ACT straight from PSUM with per-partition bias APs;
    3 DVE ops: P = h - mix*c (scalar_tensor_tensor), V = z*P,
    h' = a*c + V (scalar_tensor_tensor), a = mix + nu_s.
  * warmup for segment 0 uses r = -b_h @ U^-1 so c = tanh(0) = 0 keeps
    h exactly 0 until its true t=0.
"""

import sys

import numpy as np

sys.path.insert(0, "/opt/trn_rl_repo")

# ---- problem constants (hardcoded per contract) ----
B, T, HID, EMB, VOCAB = 256, 512, 128, 64, 32000
NCORES = 8
BC = B // NCORES  # 32 batch rows per core

# ---- tunables ----
SEGS = 8          # segments per core
WARM = 96         # warmup steps per segment
GROUPS = 2        # interleaved lane groups (latency hiding)
CHUNK = 4         # r-stream chunk, in steps
HKS = 8           # h-history super-tile steps (batched output DMA)
PHASE_MS = 0.0  # per-group initial phase shift (ms; 0 = off)
PHASE_BASE_MS = 0.0  # base offset added when phasing (must exceed pipe fill)
P_ENGINE = lambda nc: nc.vector  # engine for the P = h - mix*c op

_PROGRAM_CACHE: dict = {}
LAST_RESULTS = None
DEBUG_LABELS: dict = {}


def _sigmoid(x):
    return 1.0 / (1.0 + np.exp(-x))


def _build_program(S, Wm, G, mix, a):
    import concourse.bass as bass
    import concourse.tile as tile
    from concourse import bacc, mybir
    from concourse.tile_rust import add_dep_helper

    f32 = mybir.dt.float32
    Af = mybir.ActivationFunctionType
    Op = mybir.AluOpType

    T_seg = T // S
    nsteps = Wm + T_seg
    SG = S // G
    FD = SG * BC

    nc = bacc.Bacc(
        "TRN2",
        target_bir_lowering=False,
        debug=False,
        enable_asserts=False,
        num_devices=NCORES,
    )
    r_dram = nc.dram_tensor("r_sched", [HID, nsteps * S * BC], f32,
                            kind="ExternalInput").ap()
    u_dram = nc.dram_tensor("u_mat", [HID, HID], f32, kind="ExternalInput").ap()
    bg_dram = nc.dram_tensor("bg_col", [HID, 1], f32, kind="ExternalInput").ap()
    bh_dram = nc.dram_tensor("bh_col", [HID, 1], f32, kind="ExternalInput").ap()
    # step-major layout: col = ((j * S) + s) * BC + b for j in [0, T_seg)
    # (host de-permutes to [b, t, hid]); keeps every DMA fully contiguous
    out_dram = nc.dram_tensor("out_hist", [HID, T * BC], f32,
                              kind="ExternalOutput").ap()

    n_chunks = (nsteps + CHUNK - 1) // CHUNK

    with tile.TileContext(nc) as tc:
        with tc.tile_pool(name="const", bufs=1) as constp, \
             tc.tile_pool(name="rstream", bufs=4) as rpool, \
             tc.tile_pool(name="psum", bufs=4, space="PSUM") as pspool, \
             tc.tile_pool(name="zc", bufs=6) as zcpool, \
             tc.tile_pool(name="pv", bufs=6) as pvpool, \
             tc.tile_pool(name="hst", bufs=8) as hpool:

            u_sb = constp.tile([HID, HID], f32)
            nc.sync.dma_start(u_sb[:], u_dram)
            bg_sb = constp.tile([HID, 1], f32)
            nc.sync.dma_start(bg_sb[:], bg_dram)
            bh_sb = constp.tile([HID, 1], f32)
            nc.sync.dma_start(bh_sb[:], bh_dram)
            zeros = constp.tile([HID, FD], f32)
            nc.vector.memset(zeros[:], 0.0)



            prev = [None] * G
            hsup = [None] * G       # current h super-tile per group
            hs_j0 = [0] * G         # j_i of slice 0 of current super-tile
            hs_fl = [0] * G         # first unflushed j_i per group
            cur_r = None
            for j_i, j in enumerate(range(-Wm, T_seg)):
                prev_z = None
                prev_w = None
                if j_i % CHUNK == 0:
                    cols = min(CHUNK, nsteps - j_i) * S * BC
                    cur_r = rpool.tile([HID, CHUNK * S * BC], f32, tag="rchunk")
                    nc.gpsimd.dma_start(
                        cur_r[:, :cols],
                        r_dram[:, j_i * S * BC: j_i * S * BC + cols])
                jc = j_i % CHUNK
                for g in range(G):
                  with tc.tile_wait_until(PHASE_BASE_MS + PHASE_MS * g,
                                          enable=(PHASE_MS > 0 and j == -Wm and g > 0)):
                    c0 = (jc * S + g * SG) * BC
                    rhs_r = cur_r[:, c0: c0 + FD]
                    ps = pspool.tile([HID, FD], f32, tag="ps")
                    if j == -Wm:
                        bi = nc.tensor.matmul(ps[:], u_sb[:], rhs_r,
                                         start=True, stop=True)
                        DEBUG_LABELS[bi.ins.name] = (j, g, "mm_r")
                    else:
                        bi = nc.tensor.matmul(ps[:], u_sb[:], rhs_r,
                                         start=True, stop=False)
                        DEBUG_LABELS[bi.ins.name] = (j, g, "mm_r")
                        bi = nc.tensor.matmul(ps[:], u_sb[:], prev[g][:],
                                         start=False, stop=True)
                        DEBUG_LABELS[bi.ins.name] = (j, g, "mm_h")
                    c_t = zcpool.tile([HID, FD], f32, tag="c")
                    c_bi = nc.scalar.activation(c_t[:], ps[:], Af.Tanh,
                                                bias=bh_sb[:])
                    DEBUG_LABELS[c_bi.ins.name] = (j, g, "act_c")
                    if prev_z is not None:
                        add_dep_helper(c_bi.ins, prev_z.ins, sync=False,
                                       reason="keep ACT order c,z per group")
                    z_t = zcpool.tile([HID, FD], f32, tag="z")
                    prev_z = nc.scalar.activation(z_t[:], ps[:], Af.Sigmoid,
                                                  bias=bg_sb[:])
                    DEBUG_LABELS[prev_z.ins.name] = (j, g, "act_z")
                    hp = prev[g][:] if j > -Wm else zeros[:]
                    p_t = pvpool.tile([HID, FD], f32, tag="p")
                    p_bi = P_ENGINE(nc).scalar_tensor_tensor(
                        p_t[:], c_t[:], -mix, hp, Op.mult, Op.add)
                    DEBUG_LABELS[p_bi.ins.name] = (j, g, "dve_P")
                    if prev_w is not None:
                        add_dep_helper(p_bi.ins, prev_w.ins, sync=False,
                                       reason="keep DVE order P,V,W per group")
                    v_t = pvpool.tile([HID, FD], f32, tag="v")
                    v_bi = nc.vector.tensor_mul(v_t[:], z_t[:], p_t[:])
                    DEBUG_LABELS[v_bi.ins.name] = (j, g, "dve_V")
                    ks = j_i % HKS
                    if ks == 0:
                        h_super = hpool.tile([HID, HKS * FD], f32, tag="h")
                        hsup[g] = h_super
                        hs_j0[g] = j_i
                    h_t = hsup[g][:, ks * FD:(ks + 1) * FD]
                    prev_w = nc.vector.scalar_tensor_tensor(
                        h_t, c_t[:], a, v_t[:], Op.mult, Op.add)
                    DEBUG_LABELS[prev_w.ins.name] = (j, g, "dve_W")
                    last_super = (nsteps - 1) - hs_j0[g] < HKS
                    if (ks == HKS - 1 or j == T_seg - 1
                            or (last_super and ks == HKS // 2 - 1)):
                        # flush the >=0 unflushed suffix of this super-tile
                        jlo = max(hs_j0[g], Wm, hs_fl[g])  # first slice (j_i)
                        if j_i >= jlo:
                            k0 = jlo - hs_j0[g]
                            nk = j_i - jlo + 1
                            o0 = ((jlo - Wm) * S + g * SG) * BC
                            dst = out_dram.rearrange(
                                "p (t sb) -> p t sb", sb=S * BC)
                            dbi = nc.sync.dma_start(
                                dst[:, jlo - Wm: jlo - Wm + nk,
                                    g * SG * BC: g * SG * BC + FD],
                                hsup[g][:, k0 * FD:(k0 + nk) * FD].rearrange(
                                    "p (k f) -> p k f", f=FD))
                            DEBUG_LABELS[dbi.ins.name] = (j, g, "dma_out")
                            hs_fl[g] = j_i + 1
                    prev[g] = h_t

    nc.compile()
    return nc


def kernel(tokens, emb, W, U, b_g, b_h, zeta, nu):
    from concourse import bass_utils
    from concourse.bass_interp import get_hw_module

    tokens = np.asarray(tokens)
    emb = np.asarray(emb, dtype=np.float32)
    W = np.asarray(W, dtype=np.float32)
    U = np.asarray(U, dtype=np.float32)
    b_g = np.asarray(b_g, dtype=np.float32)
    b_h = np.asarray(b_h, dtype=np.float32)
    mix = float(_sigmoid(np.float64(np.asarray(zeta).reshape(-1)[0])))
    nu_s = float(_sigmoid(np.float64(np.asarray(nu).reshape(-1)[0])))
    a = mix + nu_s

    S, Wm, G = SEGS, WARM, GROUPS
    T_seg = T // S
    nsteps = Wm + T_seg

    # ---- host precompute: r tables ----
    Uinv = np.linalg.inv(U.astype(np.float64))
    EM = (emb.astype(np.float64) @ (W.astype(np.float64) @ Uinv)).astype(np.float32)
    rpad = (-b_h.astype(np.float64).reshape(1, HID) @ Uinv).astype(np.float32)[0]

    key = (S, Wm, G, CHUNK, round(mix, 12), round(a, 12))
    if key not in _PROGRAM_CACHE:
        _PROGRAM_CACHE[key] = _build_program(S, Wm, G, mix, a)
    nc = _PROGRAM_CACHE[key]

    # step j of segment s uses token time t = s*T_seg + j  (j in [-Wm, T_seg))
    ts_idx = (np.arange(S)[:, None] * T_seg +
              np.arange(-Wm, T_seg)[None, :])          # [S, nsteps]
    valid = ts_idx >= 0
    ts_clip = np.clip(ts_idx, 0, T - 1)

    in_maps = []
    for ci in range(NCORES):
        tok_c = tokens[ci * BC:(ci + 1) * BC]           # [BC, T]
        EMg = EM[tok_c]                                  # [BC, T, HID]
        # r_sched[p, j, s, b] = EMg[b, t(s,j), p]  (rpad where t<0)
        sched = EMg[:, ts_clip, :]                       # [BC, S, nsteps, HID]
        sched = np.where(valid[None, :, :, None], sched,
                         rpad[None, None, None, :])
        r_sched = np.ascontiguousarray(
            sched.transpose(3, 2, 1, 0)).reshape(HID, nsteps * S * BC)
        in_maps.append({
            "r_sched": r_sched.astype(np.float32),
            "u_mat": U,
            "bg_col": np.ascontiguousarray(b_g.reshape(HID, 1)),
            "bh_col": np.ascontiguousarray(b_h.reshape(HID, 1)),
        })

    old_m = nc.m
    nc.m = get_hw_module(nc.m)
    try:
        res = bass_utils.run_bass_kernel_spmd(
            nc, in_maps, core_ids=list(range(NCORES)))
    finally:
        nc.m = old_m
    global LAST_RESULTS
    LAST_RESULTS = res

    outs = []
    for ci in range(NCORES):
        oh = res.results[ci]["out_hist"]                 # [HID, T*BC]
        # layout [hid, j, s, b] -> t = s*T_seg + j
        o = oh.reshape(HID, T_seg, S, BC).transpose(3, 2, 1, 0)  # [b, s, j, hid]
        outs.append(o.reshape(BC, T, HID))
    output = np.concatenate(outs, axis=0).astype(np.float32)  # [B, T, HID]
    state = np.ascontiguousarray(output[:, -1, :])
    return output, state


# revision 30
# speedup vs baseline: 1.0006x; 1.0006x over previous
"""Trainium2 Bass kernel for nn_Encoder_21715354649023.

Math: tokens -> emb gather -> xw = x@W -> GRU-like scan over T:
    pre = xw_t + h@U ; z = sigmoid(pre+b_g) ; c = tanh(pre+b_h)
    h' = z*h + (mix*(1-z)+nu_s)*c,  mix = sigmoid(zeta), nu_s = sigmoid(nu)

Device strategy (data-parallel over batch, 32 rows/core on 8 cores):
  * r-trick: pre = (h + r_t) @ U with r_t = (xw_t) @ U^-1.  The host
    precomputes EM = emb @ (W @ U^-1) once and gathers r rows by token, so
    the per-step xw injection becomes a second accumulating matmul with the
    SAME stationary U (no PSUM preload, which DMA cannot do anyway).
  * segment parallelism: T=512 is split into S segments per core, each
    warmed up W steps from h=0 (the sigmoid gate contracts the state, so
    initial-state influence decays ~e^{-0.066 k}).  This turns 512 serial
    steps into (T/S + W) wider steps whose per-instruction overheads
    amortize across S*32 lanes.
  * per step, per lane-group: 2 matmuls (U.T@r acc U.T@h) -> PSUM;
    tanh+sigmoid on ACT straight from PSUM with per-partition bias APs;
    3 DVE ops: P = h - mix*c (scalar_tensor_tensor), V = z*P,
    h' = a*c + V (scalar_tensor_tensor), a = mix + nu_s.
  * warmup for segment 0 uses r = -b_h @ U^-1 so c = tanh(0) = 0 keeps
    h exactly 0 until its true t=0.
"""

import sys

import numpy as np

sys.path.insert(0, "/opt/trn_rl_repo")

# ---- problem constants (hardcoded per contract) ----
B, T, HID, EMB, VOCAB = 256, 512, 128, 64, 32000
NCORES = 8
BC = B // NCORES  # 32 batch rows per core

# ---- tunables ----
SEGS = 8          # segments per core
WARM = 96         # warmup steps per segment
GROUPS = 2        # interleaved lane groups (latency hiding)
CHUNK = 4         # r-stream chunk, in steps
HKS = 8           # h-history super-tile steps (batched output DMA)
PHASE_MS = 0.0  # per-group initial phase shift (ms; 0 = off)
PHASE_BASE_MS = 0.0  # base offset added when phasing (must exceed pipe fill)
P_ENGINE = lambda nc: nc.vector  # engine for the P = h - mix*c op

_PROGRAM_CACHE: dict = {}
LAST_RESULTS = None
DEBUG_LABELS: dict = {}


def _sigmoid(x):
    return 1.0 / (1.0 + np.exp(-x))


def _build_program(S, Wm, G, mix, a):
    import concourse.bass as bass
    import concourse.tile as tile
    from concourse import bacc, mybir
    from concourse.tile_rust import add_dep_helper

    f32 = mybir.dt.float32
    Af = mybir.ActivationFunctionType
    Op = mybir.AluOpType

    T_seg = T // S
    nsteps = Wm + T_seg
    SG = S // G
    FD = SG * BC

    nc = bacc.Bacc(
        "TRN2",
        target_bir_lowering=False,
        debug=False,
        enable_asserts=False,
        num_devices=NCORES,
    )
    r_dram = nc.dram_tensor("r_sched", [HID, nsteps * S * BC], f32,
                            kind="ExternalInput").ap()
    u_dram = nc.dram_tensor("u_mat", [HID, HID], f32, kind="ExternalInput").ap()
    bg_dram = nc.dram_tensor("bg_col", [HID, 1], f32, kind="ExternalInput").ap()
    bh_dram = nc.dram_tensor("bh_col", [HID, 1], f32, kind="ExternalInput").ap()
    # step-major layout: col = ((j * S) + s) * BC + b for j in [0, T_seg)
    # (host de-permutes to [b, t, hid]); keeps every DMA fully contiguous
    out_dram = nc.dram_tensor("out_hist", [HID, T * BC], f32,
                              kind="ExternalOutput").ap()

    n_chunks = (nsteps + CHUNK - 1) // CHUNK

    with tile.TileContext(nc) as tc:
        with tc.tile_pool(name="const", bufs=1) as constp, \
             tc.tile_pool(name="rstream", bufs=4) as rpool, \
             tc.tile_pool(name="psum", bufs=4, space="PSUM") as pspool, \
             tc.tile_pool(name="zc", bufs=6) as zcpool, \
             tc.tile_pool(name="pv", bufs=6) as pvpool, \
             tc.tile_pool(name="hst", bufs=12) as hpool:

            u_sb = constp.tile([HID, HID], f32)
            nc.sync.dma_start(u_sb[:], u_dram)
            bg_sb = constp.tile([HID, 1], f32)
            nc.sync.dma_start(bg_sb[:], bg_dram)
            bh_sb = constp.tile([HID, 1], f32)
            nc.sync.dma_start(bh_sb[:], bh_dram)
            zeros = constp.tile([HID, FD], f32)
            nc.vector.memset(zeros[:], 0.0)



            prev = [None] * G
            hsup = [None] * G       # current h super-tile per group
            hs_j0 = [0] * G         # j_i of slice 0 of current super-tile
            hs_fl = [0] * G         # first unflushed j_i per group
            cur_r = None
            for j_i, j in enumerate(range(-Wm, T_seg)):
                prev_z = None
                prev_w = None
                if j_i % CHUNK == 0:
                    cols = min(CHUNK, nsteps - j_i) * S * BC
                    cur_r = rpool.tile([HID, CHUNK * S * BC], f32, tag="rchunk")
                    nc.gpsimd.dma_start(
                        cur_r[:, :cols],
                        r_dram[:, j_i * S * BC: j_i * S * BC + cols])
                jc = j_i % CHUNK
                for g in range(G):
                  with tc.tile_wait_until(PHASE_BASE_MS + PHASE_MS * g,
                                          enable=(PHASE_MS > 0 and j == -Wm and g > 0)):
                    c0 = (jc * S + g * SG) * BC
                    rhs_r = cur_r[:, c0: c0 + FD]
                    ps = pspool.tile([HID, FD], f32, tag="ps")
                    if j == -Wm:
                        bi = nc.tensor.matmul(ps[:], u_sb[:], rhs_r,
                                         start=True, stop=True)
                        DEBUG_LABELS[bi.ins.name] = (j, g, "mm_r")
                    else:
                        bi = nc.tensor.matmul(ps[:], u_sb[:], rhs_r,
                                         start=True, stop=False)
                        DEBUG_LABELS[bi.ins.name] = (j, g, "mm_r")
                        bi = nc.tensor.matmul(ps[:], u_sb[:], prev[g][:],
                                         start=False, stop=True)
                        DEBUG_LABELS[bi.ins.name] = (j, g, "mm_h")
                    c_t = zcpool.tile([HID, FD], f32, tag="c")
                    c_bi = nc.scalar.activation(c_t[:], ps[:], Af.Tanh,
                                                bias=bh_sb[:])
                    DEBUG_LABELS[c_bi.ins.name] = (j, g, "act_c")
                    if prev_z is not None:
                        add_dep_helper(c_bi.ins, prev_z.ins, sync=False,
                                       reason="keep ACT order c,z per group")
                    z_t = zcpool.tile([HID, FD], f32, tag="z")
                    prev_z = nc.scalar.activation(z_t[:], ps[:], Af.Sigmoid,
                                                  bias=bg_sb[:])
                    DEBUG_LABELS[prev_z.ins.name] = (j, g, "act_z")
                    hp = prev[g][:] if j > -Wm else zeros[:]
                    p_t = pvpool.tile([HID, FD], f32, tag="p")
                    p_bi = P_ENGINE(nc).scalar_tensor_tensor(
                        p_t[:], c_t[:], -mix, hp, Op.mult, Op.add)
                    DEBUG_LABELS[p_bi.ins.name] = (j, g, "dve_P")
                    if prev_w is not None:
                        add_dep_helper(p_bi.ins, prev_w.ins, sync=False,
                                       reason="keep DVE order P,V,W per group")
                    v_t = pvpool.tile([HID, FD], f32, tag="v")
                    v_bi = nc.vector.tensor_mul(v_t[:], z_t[:], p_t[:])
                    DEBUG_LABELS[v_bi.ins.name] = (j, g, "dve_V")
                    ks = j_i % HKS
                    if ks == 0:
                        h_super = hpool.tile([HID, HKS * FD], f32, tag="h")
                        hsup[g] = h_super
                        hs_j0[g] = j_i
                    h_t = hsup[g][:, ks * FD:(ks + 1) * FD]
                    prev_w = nc.vector.scalar_tensor_tensor(
                        h_t, c_t[:], a, v_t[:], Op.mult, Op.add)
                    DEBUG_LABELS[prev_w.ins.name] = (j, g, "dve_W")
                    last_super = (nsteps - 1) - hs_j0[g] < HKS
                    if (ks == HKS - 1 or j == T_seg - 1
                            or (last_super and ks == HKS // 2 - 1)):
                        # flush the >=0 unflushed suffix of this super-tile
                        jlo = max(hs_j0[g], Wm, hs_fl[g])  # first slice (j_i)
                        if j_i >= jlo:
                            k0 = jlo - hs_j0[g]
                            nk = j_i - jlo + 1
                            o0 = ((jlo - Wm) * S + g * SG) * BC
                            dst = out_dram.rearrange(
                                "p (t sb) -> p t sb", sb=S * BC)
                            dbi = nc.sync.dma_start(
                                dst[:, jlo - Wm: jlo - Wm + nk,
                                    g * SG * BC: g * SG * BC + FD],
                                hsup[g][:, k0 * FD:(k0 + nk) * FD].rearrange(
                                    "p (k f) -> p k f", f=FD))
                            DEBUG_LABELS[dbi.ins.name] = (j, g, "dma_out")
                            hs_fl[g] = j_i + 1
                    prev[g] = h_t

    nc.compile()
    return nc


def kernel(tokens, emb, W, U, b_g, b_h, zeta, nu):
    from concourse import bass_utils
    from concourse.bass_interp import get_hw_module

    tokens = np.asarray(tokens)
    emb = np.asarray(emb, dtype=np.float32)
    W = np.asarray(W, dtype=np.float32)
    U = np.asarray(U, dtype=np.float32)
    b_g = np.asarray(b_g, dtype=np.float32)
    b_h = np.asarray(b_h, dtype=np.float32)
    mix = float(_sigmoid(np.float64(np.asarray(zeta).reshape(-1)[0])))
    nu_s = float(_sigmoid(np.float64(np.asarray(nu).reshape(-1)[0])))
    a = mix + nu_s

    S, Wm, G = SEGS, WARM, GROUPS
    T_seg = T // S
    nsteps = Wm + T_seg

    # ---- host precompute: r tables ----
    Uinv = np.linalg.inv(U.astype(np.float64))
    EM = (emb.astype(np.float64) @ (W.astype(np.float64) @ Uinv)).astype(np.float32)
    rpad = (-b_h.astype(np.float64).reshape(1, HID) @ Uinv).astype(np.float32)[0]

    key = (S, Wm, G, CHUNK, round(mix, 12), round(a, 12))
    if key not in _PROGRAM_CACHE:
        _PROGRAM_CACHE[key] = _build_program(S, Wm, G, mix, a)
    nc = _PROGRAM_CACHE[key]

    # step j of segment s uses token time t = s*T_seg + j  (j in [-Wm, T_seg))
    ts_idx = (np.arange(S)[:, None] * T_seg +
              np.arange(-Wm, T_seg)[None, :])          # [S, nsteps]
    valid = ts_idx >= 0
    ts_clip = np.clip(ts_idx, 0, T - 1)

    in_maps = []
    for ci in range(NCORES):
        tok_c = tokens[ci * BC:(ci + 1) * BC]           # [BC, T]
        EMg = EM[tok_c]                                  # [BC, T, HID]
        # r_sched[p, j, s, b] = EMg[b, t(s,j), p]  (rpad where t<0)
        sched = EMg[:, ts_clip, :]                       # [BC, S, nsteps, HID]
        sched = np.where(valid[None, :, :, None], sched,
                         rpad[None, None, None, :])
        r_sched = np.ascontiguousarray(
            sched.transpose(3, 2, 1, 0)).reshape(HID, nsteps * S * BC)
        in_maps.append({
            "r_sched": r_sched.astype(np.float32),
            "u_mat": U,
            "bg_col": np.ascontiguousarray(b_g.reshape(HID, 1)),
            "bh_col": np.ascontiguousarray(b_h.reshape(HID, 1)),
        })

    old_m = nc.m
    nc.m = get_hw_module(nc.m)
    try:
        res = bass_utils.run_bass_kernel_spmd(
            nc, in_maps, core_ids=list(range(NCORES)))
    finally:
        nc.m = old_m
    global LAST_RESULTS
    LAST_RESULTS = res

    outs = []
    for ci in range(NCORES):
        oh = res.results[ci]["out_hist"]                 # [HID, T*BC]
        # layout [hid, j, s, b] -> t = s*T_seg + j
        o = oh.reshape(HID, T_seg, S, BC).transpose(3, 2, 1, 0)  # [b, s, j, hid]
        outs.append(o.reshape(BC, T, HID))
    output = np.concatenate(outs, axis=0).astype(np.float32)  # [B, T, HID]
    state = np.ascontiguousarray(output[:, -1, :])
    return output, state
